# revision 5
# baseline (speedup 1.0000x reference)
"""PixelGAT (3-layer GATConv + mean-pool + MLP) as a Bass/Tile kernel on 8 TRN2 cores.

Sharding: nodes (and their incident edges, grouped by destination) are
partitioned into 8 contiguous blocks, one per core. The small weight
matrices are replicated. Each layer:
  - every core holds the FULL node-feature table  [N, 136] f32 in DRAM
    (row = [h + bias (128) | a_src (4) | a_dst (4)]),
  - each core processes the edges whose dst lies in its block:
    per-128-node dst group, per-128-edge tile: indirect-DMA gather of the
    src rows, one-hot scatter matmul into PSUM (segment softmax without
    max-subtraction: logits are O(1), exp cannot overflow; the softmax
    denominator is accumulated alongside the numerator in the same PSUM
    tile, and the division happens once per dst node),
  - finalize per group: divide, ELU, next layer's h/a via one matmul,
  - AllGather of the per-block table chunks forms the next layer's table.
Mean-pool partials are AllReduced, and the tiny MLP readout runs replicated.
"""
import sys
sys.path.insert(0, '/opt/trn_rl_repo')
import numpy as np
import concourse.bass as bass
import concourse.mybir as mybir
from concourse.bass_utils import run_bass_kernel_spmd
from concourse.tile import TileContext
from concourse.vector_clock import ScopedClock

# ---- problem constants (hardcoded per contract) ----
N = 100352
E = 802816
H_ = 4
C_ = 32
G_ = 2
F = 128            # H*C
W = 136            # table row width: h(128) + asrc(4) + adst(4)
NCORES = 8
BLK = N // NCORES          # 12544 nodes per core
GPC = BLK // 128           # 98 groups per core
NGRP = N // 128            # 784 groups total
NT = 11                    # random-edge tiles per group (capacity 1408 edges)
NEG = 0.2

AF = mybir.ActivationFunctionType
OP = mybir.AluOpType
DT = mybir.dt

_MAXW = 1


def _patched_drain_and_barrier(self, tick_clock, wait_clock):
    nc = self.nc
    drain_inst = nc.sync.drain()
    wait_clock.add_sem_waits(drain_inst.ins, ScopedClock({None: tick_clock.global_clock}))
    si = drain_inst.ins.sync_info
    waits = list(si.on_wait) if si and si.on_wait else []
    if len(waits) > _MAXW:
        si.on_wait.clear()
        si.on_wait.extend(waits[:_MAXW])
        rest = waits[_MAXW:]
        for i in range(0, len(rest), _MAXW):
            c = nc.sync.drain()
            csi = c.ins.sync_info
            if csi is None:
                c.ins.sync_info = mybir.SyncInfo(on_wait=list(rest[i:i+_MAXW]), on_update=[])
            else:
                csi.on_wait.extend(rest[i:i+_MAXW])
    nc.all_engine_barrier()
    assert self.sems is not None
    popped = nc._tile_sem_poison_stack.pop()
    assert popped is self._sem_poison
    nc.clear_and_free_semaphores(list(self.sems.allocated().values()))
    nc.all_engine_barrier()


TileContext._drain_and_barrier = _patched_drain_and_barrier


def _spill_excess_waits(nc, max_waits=1):
    """This walrus build allows only one sync-wait per instruction; move
    excess waits onto nops inserted just before the offending instruction."""
    for f in nc.m.functions:
        for bb in f.blocks:
            insts = bb.instructions
            i = 0
            while i < len(insts):
                inst = insts[i]
                si = inst.sync_info
                if si is None or not si.on_wait or len(si.on_wait) <= max_waits:
                    i += 1
                    continue
                waits = list(si.on_wait)
                keep = waits[-max_waits:]
                rest = waits[:-max_waits]
                si.on_wait.clear()
                si.on_wait.extend(keep)
                eng = nc.engines[inst.engine]
                pos = i
                for j in range(0, len(rest), max_waits):
                    n = eng.nop(nofuse=True, hint="spill_wait")
                    n.ins.sync_info = mybir.SyncInfo(
                        on_wait=list(rest[j:j+max_waits]), on_update=[])
                    cur_bb = nc.cur_bb.bb
                    assert cur_bb.instructions[-1] is n.ins
                    cur_bb.instructions.pop()
                    insts.insert(pos, n.ins)
                    pos += 1
                    i += 1
                i += 1


def _build_program():
    nc = bass.Bass(num_devices=NCORES)
    P = lambda n, s, d=DT.float32: nc.declare_dram_parameter(n, s, d, isOutput=False)
    xT = P("xT", [3, N])
    W0 = P("W0", [3, F]);   W0T = P("W0T", [F, 3])
    W1 = P("W1", [F, F]);   W1T = P("W1T", [F, F])
    W2 = P("W2", [F, F]);   W2T = P("W2T", [F, F])
    abd = [P(f"abd{l}", [F, 8]) for l in range(3)]
    brep = [P(f"Brep{l}", [128, W]) for l in range(3)]
    srcidx = P("srcidx", [128, GPC * NT + GPC], DT.int32)
    dstcol = P("dstcol", [128, GPC * NT], DT.uint8)
    iota = P("iota", [128, 128], DT.uint8)
    ident = P("ident", [128, 128])
    onehot = P("onehot", [128, 2 * GPC])
    invcnt = P("invcnt", [C_, G_])
    mW1 = P("mlpW1", [C_, C_]); mb1 = P("mlpb1c", [C_, 1])
    mW2 = P("mlpW2", [C_, 10]); mb2 = P("mlpb2c", [10, 1])
    yout = nc.declare_dram_parameter("y", [10, G_], DT.float32, isOutput=True)

    table0 = nc.dram_tensor("table0", [N, W], DT.float32)
    tblock1 = nc.dram_tensor("tblock1", [BLK, W], DT.float32)
    tblock2 = nc.dram_tensor("tblock2", [BLK, W], DT.float32)
    table1 = nc.dram_tensor("table1", [N, W], DT.float32, addr_space="Shared")
    table2 = nc.dram_tensor("table2", [N, W], DT.float32, addr_space="Shared")
    pool_dram = nc.dram_tensor("pool_dram", [C_, G_], DT.float32)
    pool_sh = nc.dram_tensor("pool_sh", [C_, G_], DT.float32, addr_space="Shared")
    rg = [list(range(NCORES))]

    with TileContext(nc) as tc:
        with tc.tile_pool(name="pers", bufs=1) as pers, \
             tc.tile_pool(name="sb", bufs=3) as sb, \
             tc.tile_pool(name="sS", bufs=2 * NT + 4) as sS, \
             tc.tile_pool(name="ps", bufs=3, space="PSUM") as ps, \
             tc.tile_pool(name="psacc", bufs=2, space="PSUM") as psacc, \
             tc.tile_pool(name="pspool", bufs=1, space="PSUM") as pspool:

            # ---- persistent constants ----
            def ld(parm, shape, dt=DT.float32):
                t = pers.tile(shape, dt, tag=f"pers_{parm.name}")
                nc.sync.dma_start(out=t[:], in_=parm[:])
                return t
            srct = ld(srcidx, [128, GPC * NT + GPC], DT.int32)
            dstt = ld(dstcol, [128, GPC * NT], DT.uint8)
            iot = ld(iota, [128, 128], DT.uint8)
            idt = ld(ident, [128, 128])
            oht = ld(onehot, [128, 2 * GPC])
            brt = [ld(brep[l], [128, W]) for l in range(3)]
            w0t_ = ld(W0, [3, F]); w0T_ = ld(W0T, [F, 3])
            w1t_ = ld(W1, [F, F]); w1T_ = ld(W1T, [F, F])
            w2t_ = ld(W2, [F, F]); w2T_ = ld(W2T, [F, F])
            abdt = [ld(abd[l], [F, 8]) for l in range(3)]

            # ---- Waug_l = [W_l | W_l @ att_blockdiag_l] ----
            waug = []
            for l, (wt, wT, kdim) in enumerate([(w0t_, w0T_, 3), (w1t_, w1T_, F), (w2t_, w2T_, F)]):
                wa = pers.tile([kdim, W], DT.float32, tag=f"waug{l}")
                nc.vector.tensor_copy(out=wa[:, 0:F], in_=wt[:])
                pa = ps.tile([kdim, 8], DT.float32, tag="mps", space="PSUM")
                nc.tensor.matmul(out=pa[:], lhsT=wT[:], rhs=abdt[l][:], start=True, stop=True)
                nc.vector.tensor_copy(out=wa[:, F:W], in_=pa[:])
                waug.append(wa)

            # ---- build table0 for ALL N (replicated on every core) ----
            for g in range(NGRP):
                xc = sb.tile([3, 128], DT.float32, tag="xc")
                nc.sync.dma_start(out=xc[:], in_=xT[:, g*128:(g+1)*128])
                hp = ps.tile([128, W], DT.float32, tag="mps", space="PSUM")
                nc.tensor.matmul(out=hp[:], lhsT=xc[:], rhs=waug[0][:], start=True, stop=True)
                row = sb.tile([128, W], DT.float32, tag="row")
                nc.vector.tensor_tensor(out=row[:], in0=hp[:], in1=brt[0][:], op=OP.add)
                nc.sync.dma_start(out=table0[g*128:(g+1)*128, :], in_=row[:])


            def edge_phase(l, table_in, finalize):
                for gl in range(GPC):
                    col0 = gl * NT
                    tblg = sb.tile([128, W], DT.float32, tag="tblg")
                    # NOTE: group row range is per-core: computed with partition id?
                    # SPMD same program: use per-core block offset via the fact that
                    # each core has ITS OWN tblock but table_in is full: rows depend
                    # on core id. We express core-dependent offsets through the
                    # per-core srcidx/dstcol data instead: the group rows for this
                    # core are gathered via indirect DMA with the group's self rows
                    # as indices (selfidx column NT*GPC+gl... simpler: selfsrc col)
                    nc.gpsimd.indirect_dma_start(
                        out=tblg[:], out_offset=None, in_=table_in[:],
                        in_offset=bass.IndirectOffsetOnAxis(
                            ap=srct[:, GPC*NT + gl:GPC*NT + gl + 1], axis=0))
                    Hg = sb.tile([128, NT, W], DT.float32, tag="Hg")
                    for t in range(NT):
                        nc.gpsimd.indirect_dma_start(
                            out=Hg[:, t, :], out_offset=None, in_=table_in[:],
                            in_offset=bass.IndirectOffsetOnAxis(
                                ap=srct[:, col0+t:col0+t+1], axis=0))
                    Ss = []
                    u = sb.tile([128, NT, 4], DT.float32, tag="u")
                    for t in range(NT):
                        S = sS.tile([128, 128], DT.float32, tag="S")
                        nc.vector.tensor_tensor(
                            out=S[:], in0=dstt[:, col0+t:col0+t+1].to_broadcast([128, 128]),
                            in1=iot[:], op=OP.is_equal)
                        Ss.append(S)
                        Stp = ps.tile([128, 128], DT.float32, tag="mps", space="PSUM")
                        nc.tensor.transpose(out=Stp[:], in_=S[:], identity=idt[:])
                        St = sS.tile([128, 128], DT.float32, tag="St")
                        nc.vector.tensor_copy(out=St[:], in_=Stp[:])
                        ap_ = ps.tile([128, 4], DT.float32, tag="mps", space="PSUM")
                        nc.tensor.matmul(out=ap_[:], lhsT=St[:], rhs=tblg[:, F+4:F+8],
                                         start=True, stop=True)
                        nc.vector.tensor_tensor(out=u[:, t, :], in0=Hg[:, t, F:F+4],
                                                in1=ap_[:], op=OP.add)
                    ul = sb.tile([128, NT * 4], DT.float32, tag="ul")
                    uf = u[:].rearrange("p a b -> p (a b)")
                    nc.vector.scalar_tensor_tensor(out=ul[:], in0=uf, scalar=NEG,
                                                   in1=uf, op0=OP.mult, op1=OP.max)
                    w_ = sb.tile([128, NT * 4], DT.float32, tag="w")
                    nc.scalar.activation(out=w_[:], in_=ul[:], func=AF.Exp)
                    V = sb.tile([128, NT, W - 4], DT.float32, tag="V")
                    for t in range(NT):
                        nc.vector.tensor_tensor(
                            out=V[:, t, 0:F], in0=Hg[:, t, 0:F],
                            in1=w_[:, 4*t:4*t+4].to_broadcast([128, 4, C_]), op=OP.mult)
                    nc.vector.tensor_copy(
                        out=V[:, :, F:F+4],
                        in_=w_[:].rearrange("p (a b) -> p a b", b=4))
                    scp = psacc.tile([128, W - 4], DT.float32, tag="scat", space="PSUM")
                    for t in range(NT):
                        nc.tensor.matmul(out=scp[:], lhsT=Ss[t][:], rhs=V[:, t, :],
                                         start=(t == 0), stop=(t == NT - 1))
                    # self-loops
                    usl = sb.tile([128, 4], DT.float32, tag="usl")
                    nc.vector.tensor_tensor(out=usl[:], in0=tblg[:, F:F+4],
                                            in1=tblg[:, F+4:F+8], op=OP.add)
                    usll = sb.tile([128, 4], DT.float32, tag="usll")
                    nc.vector.scalar_tensor_tensor(out=usll[:], in0=usl[:], scalar=NEG,
                                                   in1=usl[:], op0=OP.mult, op1=OP.max)
                    wsl = sb.tile([128, 4], DT.float32, tag="wsl")
                    nc.scalar.activation(out=wsl[:], in_=usll[:], func=AF.Exp)
                    Vsl = sb.tile([128, W - 4], DT.float32, tag="Vsl")
                    nc.vector.tensor_tensor(out=Vsl[:, 0:F], in0=tblg[:, 0:F],
                                            in1=wsl[:].to_broadcast([128, 4, C_]), op=OP.mult)
                    nc.vector.tensor_copy(out=Vsl[:, F:F+4], in_=wsl[:])
                    ntf = sb.tile([128, W - 4], DT.float32, tag="ntf")
                    nc.vector.tensor_tensor(out=ntf[:], in0=scp[:], in1=Vsl[:], op=OP.add)
                    r = sb.tile([128, 4], DT.float32, tag="r")
                    nc.vector.reciprocal(out=r[:], in_=ntf[:, F:F+4])
                    finalize(gl, ntf, r)

            def fin01(l):
                def fin(gl, ntf, r):
                    zq = sb.tile([128, F], DT.float32, tag="zq")
                    for h in range(H_):
                        nc.vector.tensor_scalar(
                            out=zq[:, C_*h:C_*(h+1)], in0=ntf[:, C_*h:C_*(h+1)],
                            scalar1=r[:, h:h+1], scalar2=0.0, op0=OP.mult, op1=OP.bypass)
                    em = sb.tile([128, F], DT.float32, tag="em")
                    nc.scalar.activation(out=em[:], in_=zq[:], func=AF.Exp)
                    em1m = sb.tile([128, F], DT.float32, tag="em1m")
                    nc.vector.tensor_scalar(out=em1m[:], in0=em[:], scalar1=1.0,
                                            scalar2=0.0, op0=OP.subtract, op1=OP.min)
                    z = sb.tile([128, F], DT.float32, tag="z")
                    nc.vector.scalar_tensor_tensor(out=z[:], in0=zq[:], scalar=0.0,
                                                   in1=em1m[:], op0=OP.max, op1=OP.add)
                    zTp = ps.tile([128, F], DT.float32, tag="mps", space="PSUM")
                    nc.tensor.transpose(out=zTp[:], in_=z[:], identity=idt[:])
                    zTs = sb.tile([128, F], DT.float32, tag="zTs")
                    nc.vector.tensor_copy(out=zTs[:], in_=zTp[:])
                    hp = ps.tile([128, W], DT.float32, tag="mps", space="PSUM")
                    nc.tensor.matmul(out=hp[:], lhsT=zTs[:], rhs=waug[l+1][:],
                                     start=True, stop=True)
                    row = sb.tile([128, W], DT.float32, tag="row")
                    nc.vector.tensor_tensor(out=row[:], in0=hp[:], in1=brt[l+1][:], op=OP.add)
                    tb = tblock1 if l == 0 else tblock2
                    nc.sync.dma_start(out=tb[gl*128:(gl+1)*128, :], in_=row[:])
                return fin

            pool_ps = pspool.tile([C_, G_], DT.float32, space="PSUM")

            def fin2(gl, ntf, r):
                r2 = sb.tile([128, 4], DT.float32, tag="r2")
                nc.vector.tensor_scalar(out=r2[:], in0=r[:], scalar1=0.25, scalar2=0.0,
                                        op0=OP.mult, op1=OP.bypass)
                acc = sb.tile([128, C_], DT.float32, tag="acc0")
                nc.vector.tensor_scalar(out=acc[:], in0=ntf[:, 0:C_], scalar1=r2[:, 0:1],
                                        scalar2=0.0, op0=OP.mult, op1=OP.bypass)
                for h in range(1, H_):
                    acc2 = sb.tile([128, C_], DT.float32, tag=f"acc{h}")
                    nc.vector.scalar_tensor_tensor(
                        out=acc2[:], in0=ntf[:, C_*h:C_*(h+1)], scalar=r2[:, h:h+1],
                        in1=acc[:], op0=OP.mult, op1=OP.add)
                    acc = acc2
                nc.tensor.matmul(out=pool_ps[:], lhsT=acc[:], rhs=oht[:, 2*gl:2*gl+2],
                                 start=(gl == 0), stop=(gl == GPC - 1),
                                 skip_group_check=True)

            edge_phase(0, table0, fin01(0))
            nc.gpsimd.collective_compute("AllGather", OP.bypass, replica_groups=rg,
                                         ins=[tblock1[:]], outs=[table1[:]])
            edge_phase(1, table1, fin01(1))
            nc.gpsimd.collective_compute("AllGather", OP.bypass, replica_groups=rg,
                                         ins=[tblock2[:]], outs=[table2[:]])
            edge_phase(2, table2, fin2)

            # ---- pooled mean + MLP readout (replicated) ----
            pool_sb = sb.tile([C_, G_], DT.float32, tag="poolsb")
            nc.vector.tensor_copy(out=pool_sb[:], in_=pool_ps[:])
            nc.sync.dma_start(out=pool_dram[:], in_=pool_sb[:])
            nc.gpsimd.collective_compute("AllReduce", OP.add, replica_groups=rg,
                                         ins=[pool_dram[:]], outs=[pool_sh[:]])
            psum_t = sb.tile([C_, G_], DT.float32, tag="psumt")
            nc.sync.dma_start(out=psum_t[:], in_=pool_sh[:])
            ict = pers.tile([C_, G_], DT.float32, tag="ict")
            nc.sync.dma_start(out=ict[:], in_=invcnt[:])
            mw1 = pers.tile([C_, C_], DT.float32, tag="mw1")
            nc.sync.dma_start(out=mw1[:], in_=mW1[:])
            mb1t = pers.tile([C_, 1], DT.float32, tag="mb1t")
            nc.sync.dma_start(out=mb1t[:], in_=mb1[:])
            mw2 = pers.tile([C_, 10], DT.float32, tag="mw2")
            nc.sync.dma_start(out=mw2[:], in_=mW2[:])
            mb2t = pers.tile([10, 1], DT.float32, tag="mb2t")
            nc.sync.dma_start(out=mb2t[:], in_=mb2[:])
            gT = sb.tile([C_, G_], DT.float32, tag="gT")
            nc.vector.tensor_tensor(out=gT[:], in0=psum_t[:], in1=ict[:], op=OP.mult)
            z1p = ps.tile([C_, G_], DT.float32, tag="mps", space="PSUM")
            nc.tensor.matmul(out=z1p[:], lhsT=mw1[:], rhs=gT[:], start=True, stop=True)
            z1 = sb.tile([C_, G_], DT.float32, tag="z1")
            nc.scalar.activation(out=z1[:], in_=z1p[:], func=AF.Relu, bias=mb1t[:])
            yp = ps.tile([10, G_], DT.float32, tag="mps", space="PSUM")
            nc.tensor.matmul(out=yp[:], lhsT=mw2[:], rhs=z1[:], start=True, stop=True)
            yt = sb.tile([10, G_], DT.float32, tag="yt")
            nc.scalar.activation(out=yt[:], in_=yp[:], func=AF.Identity, bias=mb2t[:])
            nc.sync.dma_start(out=yout[:], in_=yt[:])

    _spill_excess_waits(nc)
    return nc


def _block_diag_att(att_s, att_d):
    out = np.zeros((F, 8), np.float32)
    for h in range(H_):
        out[C_*h:C_*(h+1), h] = att_s[h]
        out[C_*h:C_*(h+1), 4+h] = att_d[h]
    return out


def kernel(**inputs):
    x = np.asarray(inputs['x'], np.float32)
    ei = np.asarray(inputs['edge_index']).astype(np.int64)
    batch = np.asarray(inputs['batch']).astype(np.int64)
    src_e, dst_e = ei[0], ei[1]

    # ---- host-side index preprocessing (sharding) ----
    order = np.argsort(dst_e, kind='stable')
    ss, ds = src_e[order], dst_e[order]
    gid = ds // 128
    counts = np.bincount(gid, minlength=NGRP)
    assert counts.max() <= NT * 128, f"group overflow: {counts.max()}"
    starts = np.zeros(NGRP + 1, np.int64)
    np.cumsum(counts, out=starts[1:])

    srcidx = [np.zeros((128, GPC * NT + GPC), np.int32) for _ in range(NCORES)]
    dstcol = [np.full((128, GPC * NT), 255, np.uint8) for _ in range(NCORES)]
    for g in range(NGRP):
        c, gl = divmod(g, GPC)
        e0, e1 = starts[g], starts[g + 1]
        k = e1 - e0
        j = np.arange(k)
        t, p = j // 128, j % 128
        srcidx[c][p, gl * NT + t] = ss[e0:e1]
        dstcol[c][p, gl * NT + t] = (ds[e0:e1] - g * 128).astype(np.uint8)
        # self rows of the group, used to fetch the group's own table rows
        srcidx[c][:, GPC * NT + gl] = np.arange(g * 128, (g + 1) * 128, dtype=np.int32)

    onehots = []
    for c in range(NCORES):
        oh = np.zeros((128, 2 * GPC), np.float32)
        for gl in range(GPC):
            ids = batch[(c * GPC + gl) * 128:(c * GPC + gl + 1) * 128]
            oh[np.arange(128), 2 * gl + ids] = 1.0
        onehots.append(oh)
    cnts = np.bincount(batch, minlength=G_).astype(np.float32)
    invcnt = np.tile((1.0 / np.maximum(cnts, 1.0))[None, :], (C_, 1)).astype(np.float32)

    brep0 = np.zeros((128, W), np.float32); brep0[:, 0:F] = inputs['b0'][None, :]
    brep1 = np.zeros((128, W), np.float32); brep1[:, 0:F] = inputs['b1'][None, :]
    brep2 = np.zeros((128, W), np.float32)
    brep2[:, 0:F] = np.tile(np.asarray(inputs['b2'], np.float32), H_)[None, :]

    common = {
        'xT': np.ascontiguousarray(x.T).astype(np.float32),
        'W0': np.asarray(inputs['W0'], np.float32),
        'W0T': np.ascontiguousarray(np.asarray(inputs['W0'], np.float32).T),
        'W1': np.asarray(inputs['W1'], np.float32),
        'W1T': np.ascontiguousarray(np.asarray(inputs['W1'], np.float32).T),
        'W2': np.asarray(inputs['W2'], np.float32),
        'W2T': np.ascontiguousarray(np.asarray(inputs['W2'], np.float32).T),
        'abd0': _block_diag_att(np.asarray(inputs['att_src0'], np.float32),
                                np.asarray(inputs['att_dst0'], np.float32)),
        'abd1': _block_diag_att(np.asarray(inputs['att_src1'], np.float32),
                                np.asarray(inputs['att_dst1'], np.float32)),
        'abd2': _block_diag_att(np.asarray(inputs['att_src2'], np.float32),
                                np.asarray(inputs['att_dst2'], np.float32)),
        'Brep0': brep0, 'Brep1': brep1, 'Brep2': brep2,
        'iota': np.tile(np.arange(128, dtype=np.uint8)[None, :], (128, 1)),
        'ident': np.eye(128, dtype=np.float32),
        'invcnt': invcnt,
        'mlpW1': np.asarray(inputs['mlpW1'], np.float32),
        'mlpb1c': np.asarray(inputs['mlpb1'], np.float32)[:, None],
        'mlpW2': np.asarray(inputs['mlpW2'], np.float32),
        'mlpb2c': np.asarray(inputs['mlpb2'], np.float32)[:, None],
    }
    in_maps = []
    for c in range(NCORES):
        m = dict(common)
        m['srcidx'] = srcidx[c]
        m['dstcol'] = dstcol[c]
        m['onehot'] = onehots[c]
        in_maps.append(m)

    nc = _build_program()
    res = run_bass_kernel_spmd(nc, in_maps, list(range(NCORES)))
    y = res.results[0]['y']  # [10, 2]
    return np.ascontiguousarray(y.T).astype(np.float32)


if __name__ == "__main__":
    pass


# revision 7
# speedup vs baseline: 1.1143x; 1.1143x over previous
"""PixelGAT (3-layer GATConv + mean-pool + MLP) as a Bass/Tile kernel on 8 TRN2 cores.

Sharding: nodes (and their incident edges, grouped by destination) are
partitioned into 8 contiguous blocks, one per core. The small weight
matrices are replicated. Each layer:
  - every core holds the FULL node-feature table  [N, 136] f32 in DRAM
    (row = [h + bias (128) | a_src (4) | a_dst (4)]),
  - each core processes the edges whose dst lies in its block:
    per-128-node dst group, per-128-edge tile: indirect-DMA gather of the
    src rows, one-hot scatter matmul into PSUM (segment softmax without
    max-subtraction: logits are O(1), exp cannot overflow; the softmax
    denominator is accumulated alongside the numerator in the same PSUM
    tile, and the division happens once per dst node),
  - finalize per group: divide, ELU, next layer's h/a via one matmul,
  - AllGather of the per-block table chunks forms the next layer's table.
Mean-pool partials are AllReduced, and the tiny MLP readout runs replicated.
"""
import sys
sys.path.insert(0, '/opt/trn_rl_repo')
import numpy as np
import concourse.bass as bass
import concourse.mybir as mybir
from concourse.bass_utils import run_bass_kernel_spmd
from concourse.tile import TileContext
from concourse.vector_clock import ScopedClock

# ---- problem constants (hardcoded per contract) ----
N = 100352
E = 802816
H_ = 4
C_ = 32
G_ = 2
F = 128            # H*C
W = 136            # table row width: h(128) + asrc(4) + adst(4)
NCORES = 8
BLK = N // NCORES          # 12544 nodes per core
GPC = BLK // 128           # 98 groups per core
NGRP = N // 128            # 784 groups total
NT = 11                    # random-edge tiles per group (capacity 1408 edges)
NEG = 0.2

AF = mybir.ActivationFunctionType
OP = mybir.AluOpType
DT = mybir.dt

_MAXW = 1
LAST_NC = None


def _patched_drain_and_barrier(self, tick_clock, wait_clock):
    nc = self.nc
    drain_inst = nc.sync.drain()
    wait_clock.add_sem_waits(drain_inst.ins, ScopedClock({None: tick_clock.global_clock}))
    si = drain_inst.ins.sync_info
    waits = list(si.on_wait) if si and si.on_wait else []
    if len(waits) > _MAXW:
        si.on_wait.clear()
        si.on_wait.extend(waits[:_MAXW])
        rest = waits[_MAXW:]
        for i in range(0, len(rest), _MAXW):
            c = nc.sync.drain()
            csi = c.ins.sync_info
            if csi is None:
                c.ins.sync_info = mybir.SyncInfo(on_wait=list(rest[i:i+_MAXW]), on_update=[])
            else:
                csi.on_wait.extend(rest[i:i+_MAXW])
    nc.all_engine_barrier()
    assert self.sems is not None
    popped = nc._tile_sem_poison_stack.pop()
    assert popped is self._sem_poison
    nc.clear_and_free_semaphores(list(self.sems.allocated().values()))
    nc.all_engine_barrier()


TileContext._drain_and_barrier = _patched_drain_and_barrier


def _spill_excess_waits(nc, max_waits=1):
    """This walrus build allows only one sync-wait per instruction; move
    excess waits onto nops inserted just before the offending instruction."""
    for f in nc.m.functions:
        for bb in f.blocks:
            insts = bb.instructions
            i = 0
            while i < len(insts):
                inst = insts[i]
                si = inst.sync_info
                if si is None or not si.on_wait or len(si.on_wait) <= max_waits:
                    i += 1
                    continue
                waits = list(si.on_wait)
                keep = waits[-max_waits:]
                rest = waits[:-max_waits]
                si.on_wait.clear()
                si.on_wait.extend(keep)
                eng = nc.engines[inst.engine]
                pos = i
                for j in range(0, len(rest), max_waits):
                    n = eng.nop(nofuse=True, hint="spill_wait")
                    n.ins.sync_info = mybir.SyncInfo(
                        on_wait=list(rest[j:j+max_waits]), on_update=[])
                    cur_bb = nc.cur_bb.bb
                    assert cur_bb.instructions[-1] is n.ins
                    cur_bb.instructions.pop()
                    insts.insert(pos, n.ins)
                    pos += 1
                    i += 1
                i += 1


def _build_program(nts=None):
    if nts is None:
        nts = [NT] * GPC
    cs = [0]
    for v in nts:
        cs.append(cs[-1] + v)
    totc = cs[-1]
    nc = bass.Bass(num_devices=NCORES)
    P = lambda n, s, d=DT.float32: nc.declare_dram_parameter(n, s, d, isOutput=False)
    xT = P("xT", [3, N])
    W0 = P("W0", [3, F]);   W0T = P("W0T", [F, 3])
    W1 = P("W1", [F, F]);   W1T = P("W1T", [F, F])
    W2 = P("W2", [F, F]);   W2T = P("W2T", [F, F])
    abd = [P(f"abd{l}", [F, 8]) for l in range(3)]
    brep = [P(f"Brep{l}", [128, W]) for l in range(3)]
    srcidx = P("srcidx", [128, totc + GPC], DT.int32)
    dstcol = P("dstcol", [128, totc], DT.uint8)
    iota = P("iota", [128, 128], DT.uint8)
    ident = P("ident", [128, 128])
    onehot = P("onehot", [128, 2 * GPC])
    invcnt = P("invcnt", [C_, G_])
    mW1 = P("mlpW1", [C_, C_]); mb1 = P("mlpb1c", [C_, 1])
    mW2 = P("mlpW2", [C_, 10]); mb2 = P("mlpb2c", [10, 1])
    yout = nc.declare_dram_parameter("y", [10, G_], DT.float32, isOutput=True)

    table0 = nc.dram_tensor("table0", [N, W], DT.float32)
    tblock1 = nc.dram_tensor("tblock1", [BLK, W], DT.float32)
    tblock2 = nc.dram_tensor("tblock2", [BLK, W], DT.float32)
    table1 = nc.dram_tensor("table1", [N, W], DT.float32, addr_space="Shared")
    table2 = nc.dram_tensor("table2", [N, W], DT.float32, addr_space="Shared")
    pool_dram = nc.dram_tensor("pool_dram", [C_, G_], DT.float32)
    pool_sh = nc.dram_tensor("pool_sh", [C_, G_], DT.float32, addr_space="Shared")
    rg = [list(range(NCORES))]

    with TileContext(nc) as tc:
        with tc.tile_pool(name="pers", bufs=1) as pers, \
             tc.tile_pool(name="sb", bufs=3) as sb, \
             tc.tile_pool(name="sS", bufs=2 * NT + 4) as sS, \
             tc.tile_pool(name="ps", bufs=3, space="PSUM") as ps, \
             tc.tile_pool(name="psacc", bufs=2, space="PSUM") as psacc, \
             tc.tile_pool(name="pspool", bufs=1, space="PSUM") as pspool:

            # ---- persistent constants ----
            def ld(parm, shape, dt=DT.float32):
                t = pers.tile(shape, dt, tag=f"pers_{parm.name}")
                nc.sync.dma_start(out=t[:], in_=parm[:])
                return t
            srct = ld(srcidx, [128, totc + GPC], DT.int32)
            dstt = ld(dstcol, [128, totc], DT.uint8)
            iot = ld(iota, [128, 128], DT.uint8)
            idt = ld(ident, [128, 128])
            oht = ld(onehot, [128, 2 * GPC])
            brt = [ld(brep[l], [128, W]) for l in range(3)]
            w0t_ = ld(W0, [3, F]); w0T_ = ld(W0T, [F, 3])
            w1t_ = ld(W1, [F, F]); w1T_ = ld(W1T, [F, F])
            w2t_ = ld(W2, [F, F]); w2T_ = ld(W2T, [F, F])
            abdt = [ld(abd[l], [F, 8]) for l in range(3)]

            # ---- Waug_l = [W_l | W_l @ att_blockdiag_l] ----
            waug = []
            for l, (wt, wT, kdim) in enumerate([(w0t_, w0T_, 3), (w1t_, w1T_, F), (w2t_, w2T_, F)]):
                wa = pers.tile([kdim, W], DT.float32, tag=f"waug{l}")
                nc.vector.tensor_copy(out=wa[:, 0:F], in_=wt[:])
                pa = ps.tile([kdim, 8], DT.float32, tag="mps", space="PSUM")
                nc.tensor.matmul(out=pa[:], lhsT=wT[:], rhs=abdt[l][:], start=True, stop=True)
                nc.vector.tensor_copy(out=wa[:, F:W], in_=pa[:])
                waug.append(wa)

            # ---- build table0 for ALL N (replicated on every core) ----
            for g in range(NGRP):
                xc = sb.tile([3, 128], DT.float32, tag="xc")
                nc.sync.dma_start(out=xc[:], in_=xT[:, g*128:(g+1)*128])
                hp = ps.tile([128, W], DT.float32, tag="mps", space="PSUM")
                nc.tensor.matmul(out=hp[:], lhsT=xc[:], rhs=waug[0][:], start=True, stop=True)
                row = sb.tile([128, W], DT.float32, tag="row")
                nc.vector.tensor_tensor(out=row[:], in0=hp[:], in1=brt[0][:], op=OP.add)
                nc.sync.dma_start(out=table0[g*128:(g+1)*128, :], in_=row[:])


            def edge_phase(l, table_in, finalize):
                for gl in range(GPC):
                    col0 = cs[gl]
                    nt = nts[gl]
                    tblg = sb.tile([128, W], DT.float32, tag="tblg")
                    # NOTE: group row range is per-core: computed with partition id?
                    # SPMD same program: use per-core block offset via the fact that
                    # each core has ITS OWN tblock but table_in is full: rows depend
                    # on core id. We express core-dependent offsets through the
                    # per-core srcidx/dstcol data instead: the group rows for this
                    # core are gathered via indirect DMA with the group's self rows
                    # as indices (selfidx column NT*GPC+gl... simpler: selfsrc col)
                    nc.gpsimd.indirect_dma_start(
                        out=tblg[:], out_offset=None, in_=table_in[:],
                        in_offset=bass.IndirectOffsetOnAxis(
                            ap=srct[:, totc + gl:totc + gl + 1], axis=0))
                    Hg = sb.tile([128, nt, W], DT.float32, tag="Hg")
                    for t in range(nt):
                        nc.gpsimd.indirect_dma_start(
                            out=Hg[:, t, :], out_offset=None, in_=table_in[:],
                            in_offset=bass.IndirectOffsetOnAxis(
                                ap=srct[:, col0+t:col0+t+1], axis=0))
                    Ss = []
                    u = sb.tile([128, nt, 4], DT.float32, tag="u")
                    for t in range(nt):
                        S = sS.tile([128, 128], DT.float32, tag="S")
                        nc.vector.tensor_tensor(
                            out=S[:], in0=dstt[:, col0+t:col0+t+1].to_broadcast([128, 128]),
                            in1=iot[:], op=OP.is_equal)
                        Ss.append(S)
                        Stp = ps.tile([128, 128], DT.float32, tag="mps", space="PSUM")
                        nc.tensor.transpose(out=Stp[:], in_=S[:], identity=idt[:])
                        St = sS.tile([128, 128], DT.float32, tag="St")
                        nc.vector.tensor_copy(out=St[:], in_=Stp[:])
                        ap_ = ps.tile([128, 4], DT.float32, tag="mps", space="PSUM")
                        nc.tensor.matmul(out=ap_[:], lhsT=St[:], rhs=tblg[:, F+4:F+8],
                                         start=True, stop=True)
                        nc.vector.tensor_tensor(out=u[:, t, :], in0=Hg[:, t, F:F+4],
                                                in1=ap_[:], op=OP.add)
                    ul = sb.tile([128, nt * 4], DT.float32, tag="ul")
                    uf = u[:].rearrange("p a b -> p (a b)")
                    nc.vector.scalar_tensor_tensor(out=ul[:], in0=uf, scalar=NEG,
                                                   in1=uf, op0=OP.mult, op1=OP.max)
                    w_ = sb.tile([128, nt * 4], DT.float32, tag="w")
                    nc.scalar.activation(out=w_[:], in_=ul[:], func=AF.Exp)
                    V = sb.tile([128, nt, W - 4], DT.float32, tag="V")
                    for t in range(nt):
                        nc.vector.tensor_tensor(
                            out=V[:, t, 0:F], in0=Hg[:, t, 0:F],
                            in1=w_[:, 4*t:4*t+4].to_broadcast([128, 4, C_]), op=OP.mult)
                    nc.vector.tensor_copy(
                        out=V[:, :, F:F+4],
                        in_=w_[:].rearrange("p (a b) -> p a b", b=4))
                    scp = psacc.tile([128, W - 4], DT.float32, tag="scat", space="PSUM")
                    for t in range(nt):
                        nc.tensor.matmul(out=scp[:], lhsT=Ss[t][:], rhs=V[:, t, :],
                                         start=(t == 0), stop=(t == nt - 1))
                    # self-loops
                    usl = sb.tile([128, 4], DT.float32, tag="usl")
                    nc.vector.tensor_tensor(out=usl[:], in0=tblg[:, F:F+4],
                                            in1=tblg[:, F+4:F+8], op=OP.add)
                    usll = sb.tile([128, 4], DT.float32, tag="usll")
                    nc.vector.scalar_tensor_tensor(out=usll[:], in0=usl[:], scalar=NEG,
                                                   in1=usl[:], op0=OP.mult, op1=OP.max)
                    wsl = sb.tile([128, 4], DT.float32, tag="wsl")
                    nc.scalar.activation(out=wsl[:], in_=usll[:], func=AF.Exp)
                    Vsl = sb.tile([128, W - 4], DT.float32, tag="Vsl")
                    nc.vector.tensor_tensor(out=Vsl[:, 0:F], in0=tblg[:, 0:F],
                                            in1=wsl[:].to_broadcast([128, 4, C_]), op=OP.mult)
                    nc.vector.tensor_copy(out=Vsl[:, F:F+4], in_=wsl[:])
                    ntf = sb.tile([128, W - 4], DT.float32, tag="ntf")
                    nc.vector.tensor_tensor(out=ntf[:], in0=scp[:], in1=Vsl[:], op=OP.add)
                    r = sb.tile([128, 4], DT.float32, tag="r")
                    nc.vector.reciprocal(out=r[:], in_=ntf[:, F:F+4])
                    finalize(gl, ntf, r)

            def fin01(l):
                def fin(gl, ntf, r):
                    zq = sb.tile([128, F], DT.float32, tag="zq")
                    for h in range(H_):
                        nc.vector.tensor_scalar(
                            out=zq[:, C_*h:C_*(h+1)], in0=ntf[:, C_*h:C_*(h+1)],
                            scalar1=r[:, h:h+1], scalar2=0.0, op0=OP.mult, op1=OP.bypass)
                    em = sb.tile([128, F], DT.float32, tag="em")
                    nc.scalar.activation(out=em[:], in_=zq[:], func=AF.Exp)
                    em1m = sb.tile([128, F], DT.float32, tag="em1m")
                    nc.vector.tensor_scalar(out=em1m[:], in0=em[:], scalar1=1.0,
                                            scalar2=0.0, op0=OP.subtract, op1=OP.min)
                    z = sb.tile([128, F], DT.float32, tag="z")
                    nc.vector.scalar_tensor_tensor(out=z[:], in0=zq[:], scalar=0.0,
                                                   in1=em1m[:], op0=OP.max, op1=OP.add)
                    zTp = ps.tile([128, F], DT.float32, tag="mps", space="PSUM")
                    nc.tensor.transpose(out=zTp[:], in_=z[:], identity=idt[:])
                    zTs = sb.tile([128, F], DT.float32, tag="zTs")
                    nc.vector.tensor_copy(out=zTs[:], in_=zTp[:])
                    hp = ps.tile([128, W], DT.float32, tag="mps", space="PSUM")
                    nc.tensor.matmul(out=hp[:], lhsT=zTs[:], rhs=waug[l+1][:],
                                     start=True, stop=True)
                    row = sb.tile([128, W], DT.float32, tag="row")
                    nc.vector.tensor_tensor(out=row[:], in0=hp[:], in1=brt[l+1][:], op=OP.add)
                    tb = tblock1 if l == 0 else tblock2
                    nc.sync.dma_start(out=tb[gl*128:(gl+1)*128, :], in_=row[:])
                return fin

            pool_ps = pspool.tile([C_, G_], DT.float32, space="PSUM")

            def fin2(gl, ntf, r):
                r2 = sb.tile([128, 4], DT.float32, tag="r2")
                nc.vector.tensor_scalar(out=r2[:], in0=r[:], scalar1=0.25, scalar2=0.0,
                                        op0=OP.mult, op1=OP.bypass)
                acc = sb.tile([128, C_], DT.float32, tag="acc0")
                nc.vector.tensor_scalar(out=acc[:], in0=ntf[:, 0:C_], scalar1=r2[:, 0:1],
                                        scalar2=0.0, op0=OP.mult, op1=OP.bypass)
                for h in range(1, H_):
                    acc2 = sb.tile([128, C_], DT.float32, tag=f"acc{h}")
                    nc.vector.scalar_tensor_tensor(
                        out=acc2[:], in0=ntf[:, C_*h:C_*(h+1)], scalar=r2[:, h:h+1],
                        in1=acc[:], op0=OP.mult, op1=OP.add)
                    acc = acc2
                nc.tensor.matmul(out=pool_ps[:], lhsT=acc[:], rhs=oht[:, 2*gl:2*gl+2],
                                 start=(gl == 0), stop=(gl == GPC - 1),
                                 skip_group_check=True)

            edge_phase(0, table0, fin01(0))
            nc.gpsimd.collective_compute("AllGather", OP.bypass, replica_groups=rg,
                                         ins=[tblock1[:]], outs=[table1[:]])
            edge_phase(1, table1, fin01(1))
            nc.gpsimd.collective_compute("AllGather", OP.bypass, replica_groups=rg,
                                         ins=[tblock2[:]], outs=[table2[:]])
            edge_phase(2, table2, fin2)

            # ---- pooled mean + MLP readout (replicated) ----
            pool_sb = sb.tile([C_, G_], DT.float32, tag="poolsb")
            nc.vector.tensor_copy(out=pool_sb[:], in_=pool_ps[:])
            nc.sync.dma_start(out=pool_dram[:], in_=pool_sb[:])
            nc.gpsimd.collective_compute("AllReduce", OP.add, replica_groups=rg,
                                         ins=[pool_dram[:]], outs=[pool_sh[:]])
            psum_t = sb.tile([C_, G_], DT.float32, tag="psumt")
            nc.sync.dma_start(out=psum_t[:], in_=pool_sh[:])
            ict = pers.tile([C_, G_], DT.float32, tag="ict")
            nc.sync.dma_start(out=ict[:], in_=invcnt[:])
            mw1 = pers.tile([C_, C_], DT.float32, tag="mw1")
            nc.sync.dma_start(out=mw1[:], in_=mW1[:])
            mb1t = pers.tile([C_, 1], DT.float32, tag="mb1t")
            nc.sync.dma_start(out=mb1t[:], in_=mb1[:])
            mw2 = pers.tile([C_, 10], DT.float32, tag="mw2")
            nc.sync.dma_start(out=mw2[:], in_=mW2[:])
            mb2t = pers.tile([10, 1], DT.float32, tag="mb2t")
            nc.sync.dma_start(out=mb2t[:], in_=mb2[:])
            gT = sb.tile([C_, G_], DT.float32, tag="gT")
            nc.vector.tensor_tensor(out=gT[:], in0=psum_t[:], in1=ict[:], op=OP.mult)
            z1p = ps.tile([C_, G_], DT.float32, tag="mps", space="PSUM")
            nc.tensor.matmul(out=z1p[:], lhsT=mw1[:], rhs=gT[:], start=True, stop=True)
            z1 = sb.tile([C_, G_], DT.float32, tag="z1")
            nc.scalar.activation(out=z1[:], in_=z1p[:], func=AF.Relu, bias=mb1t[:])
            yp = ps.tile([10, G_], DT.float32, tag="mps", space="PSUM")
            nc.tensor.matmul(out=yp[:], lhsT=mw2[:], rhs=z1[:], start=True, stop=True)
            yt = sb.tile([10, G_], DT.float32, tag="yt")
            nc.scalar.activation(out=yt[:], in_=yp[:], func=AF.Identity, bias=mb2t[:])
            nc.sync.dma_start(out=yout[:], in_=yt[:])

    _spill_excess_waits(nc)
    return nc


def _block_diag_att(att_s, att_d):
    out = np.zeros((F, 8), np.float32)
    for h in range(H_):
        out[C_*h:C_*(h+1), h] = att_s[h]
        out[C_*h:C_*(h+1), 4+h] = att_d[h]
    return out


def kernel(**inputs):
    x = np.asarray(inputs['x'], np.float32)
    ei = np.asarray(inputs['edge_index']).astype(np.int64)
    batch = np.asarray(inputs['batch']).astype(np.int64)
    src_e, dst_e = ei[0], ei[1]

    # ---- host-side index preprocessing (sharding) ----
    order = np.argsort(dst_e, kind='stable')
    ss, ds = src_e[order], dst_e[order]
    gid = ds // 128
    counts = np.bincount(gid, minlength=NGRP)
    starts = np.zeros(NGRP + 1, np.int64)
    np.cumsum(counts, out=starts[1:])
    nts = [max(1, int(v)) for v in
           np.ceil(counts.reshape(NCORES, GPC) / 128).astype(np.int64).max(axis=0)]
    cs = np.zeros(GPC + 1, np.int64)
    np.cumsum(nts, out=cs[1:])
    totc = int(cs[-1])

    srcidx = [np.zeros((128, totc + GPC), np.int32) for _ in range(NCORES)]
    dstcol = [np.full((128, totc), 255, np.uint8) for _ in range(NCORES)]
    for g in range(NGRP):
        c, gl = divmod(g, GPC)
        e0, e1 = starts[g], starts[g + 1]
        k = e1 - e0
        assert k <= nts[gl] * 128
        j = np.arange(k)
        t, p = j // 128, j % 128
        srcidx[c][p, cs[gl] + t] = ss[e0:e1]
        dstcol[c][p, cs[gl] + t] = (ds[e0:e1] - g * 128).astype(np.uint8)
        # self rows of the group, used to fetch the group's own table rows
        srcidx[c][:, totc + gl] = np.arange(g * 128, (g + 1) * 128, dtype=np.int32)

    onehots = []
    for c in range(NCORES):
        oh = np.zeros((128, 2 * GPC), np.float32)
        for gl in range(GPC):
            ids = batch[(c * GPC + gl) * 128:(c * GPC + gl + 1) * 128]
            oh[np.arange(128), 2 * gl + ids] = 1.0
        onehots.append(oh)
    cnts = np.bincount(batch, minlength=G_).astype(np.float32)
    invcnt = np.tile((1.0 / np.maximum(cnts, 1.0))[None, :], (C_, 1)).astype(np.float32)

    brep0 = np.zeros((128, W), np.float32); brep0[:, 0:F] = inputs['b0'][None, :]
    brep1 = np.zeros((128, W), np.float32); brep1[:, 0:F] = inputs['b1'][None, :]
    brep2 = np.zeros((128, W), np.float32)
    brep2[:, 0:F] = np.tile(np.asarray(inputs['b2'], np.float32), H_)[None, :]

    common = {
        'xT': np.ascontiguousarray(x.T).astype(np.float32),
        'W0': np.asarray(inputs['W0'], np.float32),
        'W0T': np.ascontiguousarray(np.asarray(inputs['W0'], np.float32).T),
        'W1': np.asarray(inputs['W1'], np.float32),
        'W1T': np.ascontiguousarray(np.asarray(inputs['W1'], np.float32).T),
        'W2': np.asarray(inputs['W2'], np.float32),
        'W2T': np.ascontiguousarray(np.asarray(inputs['W2'], np.float32).T),
        'abd0': _block_diag_att(np.asarray(inputs['att_src0'], np.float32),
                                np.asarray(inputs['att_dst0'], np.float32)),
        'abd1': _block_diag_att(np.asarray(inputs['att_src1'], np.float32),
                                np.asarray(inputs['att_dst1'], np.float32)),
        'abd2': _block_diag_att(np.asarray(inputs['att_src2'], np.float32),
                                np.asarray(inputs['att_dst2'], np.float32)),
        'Brep0': brep0, 'Brep1': brep1, 'Brep2': brep2,
        'iota': np.tile(np.arange(128, dtype=np.uint8)[None, :], (128, 1)),
        'ident': np.eye(128, dtype=np.float32),
        'invcnt': invcnt,
        'mlpW1': np.asarray(inputs['mlpW1'], np.float32),
        'mlpb1c': np.asarray(inputs['mlpb1'], np.float32)[:, None],
        'mlpW2': np.asarray(inputs['mlpW2'], np.float32),
        'mlpb2c': np.asarray(inputs['mlpb2'], np.float32)[:, None],
    }
    in_maps = []
    for c in range(NCORES):
        m = dict(common)
        m['srcidx'] = srcidx[c]
        m['dstcol'] = dstcol[c]
        m['onehot'] = onehots[c]
        in_maps.append(m)

    global LAST_NC
    nc = _build_program(nts)
    LAST_NC = nc
    res = run_bass_kernel_spmd(nc, in_maps, list(range(NCORES)))
    y = res.results[0]['y']  # [10, 2]
    return np.ascontiguousarray(y.T).astype(np.float32)


if __name__ == "__main__":
    pass


# revision 8
# speedup vs baseline: 1.5502x; 1.3912x over previous
"""PixelGAT (3-layer GATConv + mean-pool + MLP) as a Bass/Tile kernel on 8 TRN2 cores.

Sharding: nodes (and their incident edges, grouped by destination) are
partitioned into 8 contiguous blocks, one per core. The small weight
matrices are replicated. Each layer:
  - every core holds the FULL node-feature table  [N, 136] f32 in DRAM
    (row = [h + bias (128) | a_src (4) | a_dst (4)]),
  - each core processes the edges whose dst lies in its block:
    per-128-node dst group, per-128-edge tile: indirect-DMA gather of the
    src rows, one-hot scatter matmul into PSUM (segment softmax without
    max-subtraction: logits are O(1), exp cannot overflow; the softmax
    denominator is accumulated alongside the numerator in the same PSUM
    tile, and the division happens once per dst node),
  - finalize per group: divide, ELU, next layer's h/a via one matmul,
  - AllGather of the per-block table chunks forms the next layer's table.
Mean-pool partials are AllReduced, and the tiny MLP readout runs replicated.
"""
import sys
sys.path.insert(0, '/opt/trn_rl_repo')
import numpy as np
import concourse.bass as bass
import concourse.mybir as mybir
from concourse.bass_utils import run_bass_kernel_spmd
from concourse.tile import TileContext
from concourse.vector_clock import ScopedClock

# ---- problem constants (hardcoded per contract) ----
N = 100352
E = 802816
H_ = 4
C_ = 32
G_ = 2
F = 128            # H*C
W = 136            # table row width: h(128) + asrc(4) + adst(4)
NCORES = 8
BLK = N // NCORES          # 12544 nodes per core
GPC = BLK // 128           # 98 groups per core
NGRP = N // 128            # 784 groups total
NT = 11                    # random-edge tiles per group (capacity 1408 edges)
NEG = 0.2

AF = mybir.ActivationFunctionType
OP = mybir.AluOpType
DT = mybir.dt

_MAXW = 1
LAST_NC = None


def _patched_drain_and_barrier(self, tick_clock, wait_clock):
    nc = self.nc
    drain_inst = nc.sync.drain()
    wait_clock.add_sem_waits(drain_inst.ins, ScopedClock({None: tick_clock.global_clock}))
    si = drain_inst.ins.sync_info
    waits = list(si.on_wait) if si and si.on_wait else []
    if len(waits) > _MAXW:
        si.on_wait.clear()
        si.on_wait.extend(waits[:_MAXW])
        rest = waits[_MAXW:]
        for i in range(0, len(rest), _MAXW):
            c = nc.sync.drain()
            csi = c.ins.sync_info
            if csi is None:
                c.ins.sync_info = mybir.SyncInfo(on_wait=list(rest[i:i+_MAXW]), on_update=[])
            else:
                csi.on_wait.extend(rest[i:i+_MAXW])
    nc.all_engine_barrier()
    assert self.sems is not None
    popped = nc._tile_sem_poison_stack.pop()
    assert popped is self._sem_poison
    nc.clear_and_free_semaphores(list(self.sems.allocated().values()))
    nc.all_engine_barrier()


TileContext._drain_and_barrier = _patched_drain_and_barrier


def _spill_excess_waits(nc, max_waits=1):
    """This walrus build allows only one sync-wait per instruction; move
    excess waits onto nops inserted just before the offending instruction."""
    for f in nc.m.functions:
        for bb in f.blocks:
            insts = bb.instructions
            i = 0
            while i < len(insts):
                inst = insts[i]
                si = inst.sync_info
                if si is None or not si.on_wait or len(si.on_wait) <= max_waits:
                    i += 1
                    continue
                waits = list(si.on_wait)
                keep = waits[-max_waits:]
                rest = waits[:-max_waits]
                si.on_wait.clear()
                si.on_wait.extend(keep)
                eng = nc.engines[inst.engine]
                pos = i
                for j in range(0, len(rest), max_waits):
                    n = eng.nop(nofuse=True, hint="spill_wait")
                    n.ins.sync_info = mybir.SyncInfo(
                        on_wait=list(rest[j:j+max_waits]), on_update=[])
                    cur_bb = nc.cur_bb.bb
                    assert cur_bb.instructions[-1] is n.ins
                    cur_bb.instructions.pop()
                    insts.insert(pos, n.ins)
                    pos += 1
                    i += 1
                i += 1


def _build_program(nts=None):
    if nts is None:
        nts = [NT] * GPC
    cs = [0]
    for v in nts:
        cs.append(cs[-1] + v)
    totc = cs[-1]
    nc = bass.Bass(num_devices=NCORES)
    P = lambda n, s, d=DT.float32: nc.declare_dram_parameter(n, s, d, isOutput=False)
    xT = P("xT", [3, N])
    W0 = P("W0", [3, F]);   W0T = P("W0T", [F, 3])
    W1 = P("W1", [F, F]);   W1T = P("W1T", [F, F])
    W2 = P("W2", [F, F]);   W2T = P("W2T", [F, F])
    abd = [P(f"abd{l}", [F, 8]) for l in range(3)]
    brep = [P(f"Brep{l}", [128, W]) for l in range(3)]
    srcidx = P("srcidx", [128, totc + GPC], DT.int32)
    dstcol = P("dstcol", [128, totc], DT.uint8)
    iota = P("iota", [128, 128], DT.uint8)
    ident = P("ident", [128, 128])
    onehot = P("onehot", [128, 2 * GPC])
    invcnt = P("invcnt", [C_, G_])
    mW1 = P("mlpW1", [C_, C_]); mb1 = P("mlpb1c", [C_, 1])
    mW2 = P("mlpW2", [C_, 10]); mb2 = P("mlpb2c", [10, 1])
    yout = nc.declare_dram_parameter("y", [10, G_], DT.float32, isOutput=True)

    table0 = nc.dram_tensor("table0", [N, W], DT.float32)
    tblock1 = nc.dram_tensor("tblock1", [BLK, W], DT.float32)
    tblock2 = nc.dram_tensor("tblock2", [BLK, W], DT.float32)
    table1 = nc.dram_tensor("table1", [N, W], DT.float32, addr_space="Shared")
    table2 = nc.dram_tensor("table2", [N, W], DT.float32, addr_space="Shared")
    pool_dram = nc.dram_tensor("pool_dram", [C_, G_], DT.float32)
    pool_sh = nc.dram_tensor("pool_sh", [C_, G_], DT.float32, addr_space="Shared")
    rg = [list(range(NCORES))]

    with TileContext(nc) as tc:
        with tc.tile_pool(name="pers", bufs=1) as pers, \
             tc.tile_pool(name="sb", bufs=3) as sb, \
             tc.tile_pool(name="sS", bufs=2 * NT + 4) as sS, \
             tc.tile_pool(name="ps", bufs=1, space="PSUM") as ps, \
             tc.tile_pool(name="psT", bufs=2, space="PSUM") as psT, \
             tc.tile_pool(name="psA", bufs=2, space="PSUM") as psA, \
             tc.tile_pool(name="psacc", bufs=2, space="PSUM") as psacc, \
             tc.tile_pool(name="pspool", bufs=1, space="PSUM") as pspool:

            # ---- persistent constants ----
            def ld(parm, shape, dt=DT.float32):
                t = pers.tile(shape, dt, tag=f"pers_{parm.name}")
                nc.sync.dma_start(out=t[:], in_=parm[:])
                return t
            srct = ld(srcidx, [128, totc + GPC], DT.int32)
            dstt = ld(dstcol, [128, totc], DT.uint8)
            iot = ld(iota, [128, 128], DT.uint8)
            idt = ld(ident, [128, 128])
            oht = ld(onehot, [128, 2 * GPC])
            brt = [ld(brep[l], [128, W]) for l in range(3)]
            w0t_ = ld(W0, [3, F]); w0T_ = ld(W0T, [F, 3])
            w1t_ = ld(W1, [F, F]); w1T_ = ld(W1T, [F, F])
            w2t_ = ld(W2, [F, F]); w2T_ = ld(W2T, [F, F])
            abdt = [ld(abd[l], [F, 8]) for l in range(3)]

            # ---- Waug_l = [W_l | W_l @ att_blockdiag_l] ----
            waug = []
            for l, (wt, wT, kdim) in enumerate([(w0t_, w0T_, 3), (w1t_, w1T_, F), (w2t_, w2T_, F)]):
                wa = pers.tile([kdim, W], DT.float32, tag=f"waug{l}")
                nc.vector.tensor_copy(out=wa[:, 0:F], in_=wt[:])
                pa = ps.tile([kdim, 8], DT.float32, tag="mps", space="PSUM")
                nc.tensor.matmul(out=pa[:], lhsT=wT[:], rhs=abdt[l][:], start=True, stop=True)
                nc.vector.tensor_copy(out=wa[:, F:W], in_=pa[:])
                waug.append(wa)

            # ---- build table0 for ALL N (replicated on every core) ----
            for g in range(NGRP):
                xc = sb.tile([3, 128], DT.float32, tag="xc")
                nc.sync.dma_start(out=xc[:], in_=xT[:, g*128:(g+1)*128])
                hp = ps.tile([128, W], DT.float32, tag="mps", space="PSUM")
                nc.tensor.matmul(out=hp[:], lhsT=xc[:], rhs=waug[0][:], start=True, stop=True)
                row = sb.tile([128, W], DT.float32, tag="row")
                nc.vector.tensor_tensor(out=row[:], in0=hp[:], in1=brt[0][:], op=OP.add)
                nc.sync.dma_start(out=table0[g*128:(g+1)*128, :], in_=row[:])


            def edge_phase(l, table_in, finalize):
                for gl in range(GPC):
                    col0 = cs[gl]
                    nt = nts[gl]
                    tblg = sb.tile([128, W], DT.float32, tag="tblg")
                    # NOTE: group row range is per-core: computed with partition id?
                    # SPMD same program: use per-core block offset via the fact that
                    # each core has ITS OWN tblock but table_in is full: rows depend
                    # on core id. We express core-dependent offsets through the
                    # per-core srcidx/dstcol data instead: the group rows for this
                    # core are gathered via indirect DMA with the group's self rows
                    # as indices (selfidx column NT*GPC+gl... simpler: selfsrc col)
                    nc.gpsimd.indirect_dma_start(
                        out=tblg[:], out_offset=None, in_=table_in[:],
                        in_offset=bass.IndirectOffsetOnAxis(
                            ap=srct[:, totc + gl:totc + gl + 1], axis=0))
                    Hg = sb.tile([128, nt, W], DT.float32, tag="Hg")
                    for t in range(nt):
                        nc.gpsimd.indirect_dma_start(
                            out=Hg[:, t, :], out_offset=None, in_=table_in[:],
                            in_offset=bass.IndirectOffsetOnAxis(
                                ap=srct[:, col0+t:col0+t+1], axis=0))
                    Ss = []
                    u = sb.tile([128, nt, 4], DT.float32, tag="u")
                    for t in range(nt):
                        S = sS.tile([128, 128], DT.float32, tag="S")
                        nc.vector.tensor_tensor(
                            out=S[:], in0=dstt[:, col0+t:col0+t+1].to_broadcast([128, 128]),
                            in1=iot[:], op=OP.is_equal)
                        Ss.append(S)
                        Stp = psT.tile([128, 128], DT.float32, tag="Stp", space="PSUM")
                        nc.tensor.transpose(out=Stp[:], in_=S[:], identity=idt[:])
                        St = sS.tile([128, 128], DT.float32, tag="St")
                        nc.scalar.copy(out=St[:], in_=Stp[:])
                        ap_ = psA.tile([128, 4], DT.float32, tag="adst", space="PSUM")
                        nc.tensor.matmul(out=ap_[:], lhsT=St[:], rhs=tblg[:, F+4:F+8],
                                         start=True, stop=True)
                        nc.vector.tensor_tensor(out=u[:, t, :], in0=Hg[:, t, F:F+4],
                                                in1=ap_[:], op=OP.add)
                    ul = sb.tile([128, nt * 4], DT.float32, tag="ul")
                    uf = u[:].rearrange("p a b -> p (a b)")
                    nc.vector.scalar_tensor_tensor(out=ul[:], in0=uf, scalar=NEG,
                                                   in1=uf, op0=OP.mult, op1=OP.max)
                    w_ = sb.tile([128, nt * 4], DT.float32, tag="w")
                    nc.scalar.activation(out=w_[:], in_=ul[:], func=AF.Exp)
                    V = sb.tile([128, nt, W - 4], DT.float32, tag="V")
                    for t in range(nt):
                        nc.vector.tensor_tensor(
                            out=V[:, t, 0:F], in0=Hg[:, t, 0:F],
                            in1=w_[:, 4*t:4*t+4].to_broadcast([128, 4, C_]), op=OP.mult)
                    nc.vector.tensor_copy(
                        out=V[:, :, F:F+4],
                        in_=w_[:].rearrange("p (a b) -> p a b", b=4))
                    scp = psacc.tile([128, W - 4], DT.float32, tag="scat", space="PSUM")
                    for t in range(nt):
                        nc.tensor.matmul(out=scp[:], lhsT=Ss[t][:], rhs=V[:, t, :],
                                         start=(t == 0), stop=(t == nt - 1))
                    # self-loops
                    usl = sb.tile([128, 4], DT.float32, tag="usl")
                    nc.vector.tensor_tensor(out=usl[:], in0=tblg[:, F:F+4],
                                            in1=tblg[:, F+4:F+8], op=OP.add)
                    usll = sb.tile([128, 4], DT.float32, tag="usll")
                    nc.vector.scalar_tensor_tensor(out=usll[:], in0=usl[:], scalar=NEG,
                                                   in1=usl[:], op0=OP.mult, op1=OP.max)
                    wsl = sb.tile([128, 4], DT.float32, tag="wsl")
                    nc.scalar.activation(out=wsl[:], in_=usll[:], func=AF.Exp)
                    Vsl = sb.tile([128, W - 4], DT.float32, tag="Vsl")
                    nc.vector.tensor_tensor(out=Vsl[:, 0:F], in0=tblg[:, 0:F],
                                            in1=wsl[:].to_broadcast([128, 4, C_]), op=OP.mult)
                    nc.vector.tensor_copy(out=Vsl[:, F:F+4], in_=wsl[:])
                    ntf = sb.tile([128, W - 4], DT.float32, tag="ntf")
                    nc.vector.tensor_tensor(out=ntf[:], in0=scp[:], in1=Vsl[:], op=OP.add)
                    r = sb.tile([128, 4], DT.float32, tag="r")
                    nc.vector.reciprocal(out=r[:], in_=ntf[:, F:F+4])
                    finalize(gl, ntf, r)

            def fin01(l):
                def fin(gl, ntf, r):
                    zq = sb.tile([128, F], DT.float32, tag="zq")
                    for h in range(H_):
                        nc.vector.tensor_scalar(
                            out=zq[:, C_*h:C_*(h+1)], in0=ntf[:, C_*h:C_*(h+1)],
                            scalar1=r[:, h:h+1], scalar2=0.0, op0=OP.mult, op1=OP.bypass)
                    em = sb.tile([128, F], DT.float32, tag="em")
                    nc.scalar.activation(out=em[:], in_=zq[:], func=AF.Exp)
                    em1m = sb.tile([128, F], DT.float32, tag="em1m")
                    nc.vector.tensor_scalar(out=em1m[:], in0=em[:], scalar1=1.0,
                                            scalar2=0.0, op0=OP.subtract, op1=OP.min)
                    z = sb.tile([128, F], DT.float32, tag="z")
                    nc.vector.scalar_tensor_tensor(out=z[:], in0=zq[:], scalar=0.0,
                                                   in1=em1m[:], op0=OP.max, op1=OP.add)
                    zTp = ps.tile([128, F], DT.float32, tag="mps", space="PSUM")
                    nc.tensor.transpose(out=zTp[:], in_=z[:], identity=idt[:])
                    zTs = sb.tile([128, F], DT.float32, tag="zTs")
                    nc.vector.tensor_copy(out=zTs[:], in_=zTp[:])
                    hp = ps.tile([128, W], DT.float32, tag="mps", space="PSUM")
                    nc.tensor.matmul(out=hp[:], lhsT=zTs[:], rhs=waug[l+1][:],
                                     start=True, stop=True)
                    row = sb.tile([128, W], DT.float32, tag="row")
                    nc.vector.tensor_tensor(out=row[:], in0=hp[:], in1=brt[l+1][:], op=OP.add)
                    tb = tblock1 if l == 0 else tblock2
                    nc.sync.dma_start(out=tb[gl*128:(gl+1)*128, :], in_=row[:])
                return fin

            pool_ps = pspool.tile([C_, G_], DT.float32, space="PSUM")

            def fin2(gl, ntf, r):
                r2 = sb.tile([128, 4], DT.float32, tag="r2")
                nc.vector.tensor_scalar(out=r2[:], in0=r[:], scalar1=0.25, scalar2=0.0,
                                        op0=OP.mult, op1=OP.bypass)
                acc = sb.tile([128, C_], DT.float32, tag="acc0")
                nc.vector.tensor_scalar(out=acc[:], in0=ntf[:, 0:C_], scalar1=r2[:, 0:1],
                                        scalar2=0.0, op0=OP.mult, op1=OP.bypass)
                for h in range(1, H_):
                    acc2 = sb.tile([128, C_], DT.float32, tag=f"acc{h}")
                    nc.vector.scalar_tensor_tensor(
                        out=acc2[:], in0=ntf[:, C_*h:C_*(h+1)], scalar=r2[:, h:h+1],
                        in1=acc[:], op0=OP.mult, op1=OP.add)
                    acc = acc2
                nc.tensor.matmul(out=pool_ps[:], lhsT=acc[:], rhs=oht[:, 2*gl:2*gl+2],
                                 start=(gl == 0), stop=(gl == GPC - 1),
                                 skip_group_check=True)

            edge_phase(0, table0, fin01(0))
            nc.gpsimd.collective_compute("AllGather", OP.bypass, replica_groups=rg,
                                         ins=[tblock1[:]], outs=[table1[:]])
            edge_phase(1, table1, fin01(1))
            nc.gpsimd.collective_compute("AllGather", OP.bypass, replica_groups=rg,
                                         ins=[tblock2[:]], outs=[table2[:]])
            edge_phase(2, table2, fin2)

            # ---- pooled mean + MLP readout (replicated) ----
            pool_sb = sb.tile([C_, G_], DT.float32, tag="poolsb")
            nc.vector.tensor_copy(out=pool_sb[:], in_=pool_ps[:])
            nc.sync.dma_start(out=pool_dram[:], in_=pool_sb[:])
            nc.gpsimd.collective_compute("AllReduce", OP.add, replica_groups=rg,
                                         ins=[pool_dram[:]], outs=[pool_sh[:]])
            psum_t = sb.tile([C_, G_], DT.float32, tag="psumt")
            nc.sync.dma_start(out=psum_t[:], in_=pool_sh[:])
            ict = pers.tile([C_, G_], DT.float32, tag="ict")
            nc.sync.dma_start(out=ict[:], in_=invcnt[:])
            mw1 = pers.tile([C_, C_], DT.float32, tag="mw1")
            nc.sync.dma_start(out=mw1[:], in_=mW1[:])
            mb1t = pers.tile([C_, 1], DT.float32, tag="mb1t")
            nc.sync.dma_start(out=mb1t[:], in_=mb1[:])
            mw2 = pers.tile([C_, 10], DT.float32, tag="mw2")
            nc.sync.dma_start(out=mw2[:], in_=mW2[:])
            mb2t = pers.tile([10, 1], DT.float32, tag="mb2t")
            nc.sync.dma_start(out=mb2t[:], in_=mb2[:])
            gT = sb.tile([C_, G_], DT.float32, tag="gT")
            nc.vector.tensor_tensor(out=gT[:], in0=psum_t[:], in1=ict[:], op=OP.mult)
            z1p = ps.tile([C_, G_], DT.float32, tag="mps", space="PSUM")
            nc.tensor.matmul(out=z1p[:], lhsT=mw1[:], rhs=gT[:], start=True, stop=True)
            z1 = sb.tile([C_, G_], DT.float32, tag="z1")
            nc.scalar.activation(out=z1[:], in_=z1p[:], func=AF.Relu, bias=mb1t[:])
            yp = ps.tile([10, G_], DT.float32, tag="mps", space="PSUM")
            nc.tensor.matmul(out=yp[:], lhsT=mw2[:], rhs=z1[:], start=True, stop=True)
            yt = sb.tile([10, G_], DT.float32, tag="yt")
            nc.scalar.activation(out=yt[:], in_=yp[:], func=AF.Identity, bias=mb2t[:])
            nc.sync.dma_start(out=yout[:], in_=yt[:])

    _spill_excess_waits(nc)
    return nc


def _block_diag_att(att_s, att_d):
    out = np.zeros((F, 8), np.float32)
    for h in range(H_):
        out[C_*h:C_*(h+1), h] = att_s[h]
        out[C_*h:C_*(h+1), 4+h] = att_d[h]
    return out


def kernel(**inputs):
    x = np.asarray(inputs['x'], np.float32)
    ei = np.asarray(inputs['edge_index']).astype(np.int64)
    batch = np.asarray(inputs['batch']).astype(np.int64)
    src_e, dst_e = ei[0], ei[1]

    # ---- host-side index preprocessing (sharding) ----
    order = np.argsort(dst_e, kind='stable')
    ss, ds = src_e[order], dst_e[order]
    gid = ds // 128
    counts = np.bincount(gid, minlength=NGRP)
    starts = np.zeros(NGRP + 1, np.int64)
    np.cumsum(counts, out=starts[1:])
    nts = [max(1, int(v)) for v in
           np.ceil(counts.reshape(NCORES, GPC) / 128).astype(np.int64).max(axis=0)]
    cs = np.zeros(GPC + 1, np.int64)
    np.cumsum(nts, out=cs[1:])
    totc = int(cs[-1])

    srcidx = [np.zeros((128, totc + GPC), np.int32) for _ in range(NCORES)]
    dstcol = [np.full((128, totc), 255, np.uint8) for _ in range(NCORES)]
    for g in range(NGRP):
        c, gl = divmod(g, GPC)
        e0, e1 = starts[g], starts[g + 1]
        k = e1 - e0
        assert k <= nts[gl] * 128
        j = np.arange(k)
        t, p = j // 128, j % 128
        srcidx[c][p, cs[gl] + t] = ss[e0:e1]
        dstcol[c][p, cs[gl] + t] = (ds[e0:e1] - g * 128).astype(np.uint8)
        # self rows of the group, used to fetch the group's own table rows
        srcidx[c][:, totc + gl] = np.arange(g * 128, (g + 1) * 128, dtype=np.int32)

    onehots = []
    for c in range(NCORES):
        oh = np.zeros((128, 2 * GPC), np.float32)
        for gl in range(GPC):
            ids = batch[(c * GPC + gl) * 128:(c * GPC + gl + 1) * 128]
            oh[np.arange(128), 2 * gl + ids] = 1.0
        onehots.append(oh)
    cnts = np.bincount(batch, minlength=G_).astype(np.float32)
    invcnt = np.tile((1.0 / np.maximum(cnts, 1.0))[None, :], (C_, 1)).astype(np.float32)

    brep0 = np.zeros((128, W), np.float32); brep0[:, 0:F] = inputs['b0'][None, :]
    brep1 = np.zeros((128, W), np.float32); brep1[:, 0:F] = inputs['b1'][None, :]
    brep2 = np.zeros((128, W), np.float32)
    brep2[:, 0:F] = np.tile(np.asarray(inputs['b2'], np.float32), H_)[None, :]

    common = {
        'xT': np.ascontiguousarray(x.T).astype(np.float32),
        'W0': np.asarray(inputs['W0'], np.float32),
        'W0T': np.ascontiguousarray(np.asarray(inputs['W0'], np.float32).T),
        'W1': np.asarray(inputs['W1'], np.float32),
        'W1T': np.ascontiguousarray(np.asarray(inputs['W1'], np.float32).T),
        'W2': np.asarray(inputs['W2'], np.float32),
        'W2T': np.ascontiguousarray(np.asarray(inputs['W2'], np.float32).T),
        'abd0': _block_diag_att(np.asarray(inputs['att_src0'], np.float32),
                                np.asarray(inputs['att_dst0'], np.float32)),
        'abd1': _block_diag_att(np.asarray(inputs['att_src1'], np.float32),
                                np.asarray(inputs['att_dst1'], np.float32)),
        'abd2': _block_diag_att(np.asarray(inputs['att_src2'], np.float32),
                                np.asarray(inputs['att_dst2'], np.float32)),
        'Brep0': brep0, 'Brep1': brep1, 'Brep2': brep2,
        'iota': np.tile(np.arange(128, dtype=np.uint8)[None, :], (128, 1)),
        'ident': np.eye(128, dtype=np.float32),
        'invcnt': invcnt,
        'mlpW1': np.asarray(inputs['mlpW1'], np.float32),
        'mlpb1c': np.asarray(inputs['mlpb1'], np.float32)[:, None],
        'mlpW2': np.asarray(inputs['mlpW2'], np.float32),
        'mlpb2c': np.asarray(inputs['mlpb2'], np.float32)[:, None],
    }
    in_maps = []
    for c in range(NCORES):
        m = dict(common)
        m['srcidx'] = srcidx[c]
        m['dstcol'] = dstcol[c]
        m['onehot'] = onehots[c]
        in_maps.append(m)

    global LAST_NC
    nc = _build_program(nts)
    LAST_NC = nc
    res = run_bass_kernel_spmd(nc, in_maps, list(range(NCORES)))
    y = res.results[0]['y']  # [10, 2]
    return np.ascontiguousarray(y.T).astype(np.float32)


if __name__ == "__main__":
    pass
